# revision 30
# baseline (speedup 1.0000x reference)
"""Trainium2 Bass kernel for nn_DepatchSampling.

Strategy (hardcoded for B=32, C=64, L=4096, PS=16, STRIDE=8, PC=511, HID=64):

 - Pure data parallelism: batch dim (32) sharded over 8 cores, 4 batches each.
 - Per core, 256 (b,c) rows in 2 chunks of 128 rows (one row per partition).
 - Everything datapath-heavy runs in bf16 (validated: rel err ~2.8e-3 vs the
   2e-2 gate):
     * X is DMA'd in as bf16 twice: once row-major (xsb), once transposed
       into L-major 128-blocks via the DMA xbar (xt) — no PE transposes.
     * conv1 runs as bf16 matmuls (1 cyc/row), 12 patch-pairs per PSUM tile;
       gelu(+b1) on ACT; conv2 as tiny bf16 matmuls producing [row,
       (patch, j)] directly in PSUM.
 - Box decode (per patch sub-range, on DVE): ds = relu(o1+b2[1]+7.5);
   an = dx+b2[0]+8p+7.5; lo/hi = clip(an -/+ ds, 0, 4095);
   alpha = lo-8p, beta = (hi-lo)/15 - 1.
 - Sampling identity: with j = 8p+s and w = alpha + beta*s in [-1, 1):
       out = X[j] + w*D1[j-1] + relu(w)*D2[j],
   D1[i] = X[i]-X[i-1] (shifted), D2[j] = D1[j+1]-D1[j].  All accesses are
   static strided views.  w is built as 16 per-s strided slabs
   (w[:, s::16] = beta*s + alpha, one STT per s).  The packed bf16 ops
   (relu/mult/mult/add/add) are split between DVE (2x/4x modes) and GPSIMD
   for engine balance.  Decode/interp runs on progressively finer sub-units
   at the start and end of the schedule to shorten pipeline fill/drain.
 - Output stored bf16, upcast on host.
"""

import numpy as np
import ml_dtypes

import concourse.bass as bass
import concourse.bacc as bacc
import concourse.mybir as mybir
from concourse.tile import TileContext
from concourse.bass_utils import run_bass_kernel_spmd

F32 = mybir.dt.float32
BF16 = mybir.dt.bfloat16
AF = mybir.ActivationFunctionType
OP = mybir.AluOpType
NPBF = ml_dtypes.bfloat16

# Problem constants
B, C, L = 32, 64, 4096
PS, STRIDE, PC, HID = 16, 8, 511, 64
NCORES = 8
BPC = B // NCORES            # batches per core
ROWS = BPC * C               # 256 (b,c) rows per core
NCHUNK = 2                   # chunks of 128 rows
NT = 256                     # patch-pair index t per chunk: p = 2t, 2t+1
TBLK = 12                    # t per conv1 PSUM tile (conv unit)
XOFF = 8                     # X[j] lives at xsb[:, XOFF + j]
XW = XOFF + L + 8            # padded row width

# decode/interp sub-unit boundaries (in patches within a 256-patch pair),
# per (chunk, pair): finer at schedule start (early GPSIMD fill) and end
# (short drain).
SUBS = {
    (0, 0): [0, 64, 128, 256],
    (0, 1): [0, 256],
    (1, 0): [0, 256],
    (1, 1): [0, 128, 192, 256],
}

_CACHE = {}


def _consts(W1, b1, W2, b2):
    """Host-side packing of weights and constant tables."""
    W1 = np.asarray(W1, np.float32)
    b1 = np.asarray(b1, np.float32)
    W2 = np.asarray(W2, np.float32)
    b2 = np.asarray(b2, np.float32)

    # conv1 weight packs: pair t covers rows [16t, 16t+24) of the L axis;
    # within its 128-block the pair sits at row offset rho = 16*(t mod 8).
    # rho <= 96: single matmul with W1R{rho}; rho == 112: split into W1SA on
    # block A plus W1SB on block A+1, accumulated in PSUM.
    tens = {}
    for rho in range(0, 112, 16):
        full = np.zeros((128, 128), np.float32)
        full[rho:rho + 16, 0:64] = W1.T
        full[rho + 8:rho + 24, 64:128] = W1.T
        tens[f"W1R{rho}"] = full.astype(NPBF)
    w1sa = np.zeros((128, 128), np.float32)
    w1sa[112:128, 0:64] = W1.T
    w1sa[120:128, 64:128] = W1.T[0:8]       # odd patch s = 0..7
    tens["W1SA"] = w1sa.astype(NPBF)
    w1sb = np.zeros((128, 128), np.float32)
    w1sb[0:8, 64:128] = W1.T[8:16]          # odd patch s = 8..15
    tens["W1SB"] = w1sb.astype(NPBF)

    w2p = np.zeros((128, 4), np.float32)
    w2p[0:64, 0] = W2[0]
    w2p[0:64, 1] = W2[1]
    w2p[64:128, 2] = W2[0]
    w2p[64:128, 3] = W2[1]
    tens["W2P"] = w2p.astype(NPBF)
    tens["B1P"] = np.concatenate([b1, b1]).reshape(128, 1).astype(np.float32)

    p = np.arange(512, dtype=np.float32)
    tens["AREP"] = np.broadcast_to(8 * p + np.float32(7.5), (128, 512)).copy()
    tens["P8N"] = np.broadcast_to(-8 * p, (128, 512)).copy()

    # batch all constants into one bf16 + one f32 DMA (HWDGE dispatch is
    # serial at ~650ns each — 15 separate loads would stall startup)
    cb = np.concatenate([tens[k] for k in BF_ORDER], axis=1)
    cf = np.concatenate([tens[k].astype(np.float32) for k in F32_ORDER],
                        axis=1)
    packed = {"CB": np.ascontiguousarray(cb.astype(NPBF)),
              "CF": np.ascontiguousarray(cf.astype(np.float32))}

    scal = {
        "c_ds": float(np.float32(b2[1]) + np.float32(7.5)),
        "b20": float(np.float32(b2[0])),
    }
    return packed, scal


BF_ORDER = ([f"W1R{rho}" for rho in range(0, 112, 16)] +
            ["W1SA", "W1SB", "W2P"])
BF_COLS = [128] * 9 + [4]
F32_ORDER = ["B1P", "AREP", "P8N"]
F32_COLS = [1, 512, 512]


def _ap(tile_ap, col_off, dims):
    """Custom strided view of a 2D [128, F] tile: dims = [[step, count], ...]
    appended after the partition dim."""
    pstep = tile_ap.ap[0][0]
    npart = tile_ap.ap[0][1]
    return bass.AP(tile_ap.tensor, tile_ap.offset + col_off,
                   [[pstep, npart]] + [list(d) for d in dims])


def build(scal):
    nc = bacc.Bacc("TRN2", target_bir_lowering=False, debug=False)

    XS = nc.dram_tensor("XS", [ROWS, L], BF16, kind="ExternalInput")
    OUT = nc.dram_tensor("OUT", [ROWS, PC * PS], BF16, kind="ExternalOutput")
    CB = nc.dram_tensor("CB", [128, sum(BF_COLS)], BF16, kind="ExternalInput")
    CF = nc.dram_tensor("CF", [128, sum(F32_COLS)], F32, kind="ExternalInput")

    c_ds, b20 = scal["c_ds"], scal["b20"]

    with TileContext(nc) as tc:
        with tc.tile_pool(name="consts", bufs=1) as cpool, \
             tc.tile_pool(name="xbig", bufs=2) as xpool, \
             tc.tile_pool(name="work", bufs=2) as wpool, \
             tc.tile_pool(name="psum", bufs=2, space="PSUM") as ppool:

            xsbs, xts, d1ts, d2ts = [], [], [], []
            # ---- phase 1a: weights first, then chunk-0 X (HWDGE serial) ----
            cbt = cpool.tile([128, sum(BF_COLS)], BF16, tag="c_CB")
            nc.sync.dma_start(cbt[:, :], CB[:, :])
            cft = cpool.tile([128, sum(F32_COLS)], F32, tag="c_CF")
            ccol = {}
            c0 = 0
            for k, w in zip(BF_ORDER, BF_COLS):
                ccol[k] = (cbt, c0, w)
                c0 += w
            c0 = 0
            for k, w in zip(F32_ORDER, F32_COLS):
                ccol[k] = (cft, c0, w)
                c0 += w

            def cview(k, p0=0, pn=128, col0=0, ncols=None):
                tile, base, w = ccol[k]
                ap = tile[:, :]
                pstep = ap.ap[0][0]
                return bass.AP(ap.tensor,
                               ap.offset + p0 * pstep + base + col0,
                               [[pstep, pn],
                                [1, ncols if ncols is not None else w]])

            for chunk in range(NCHUNK):
                r0 = chunk * 128
                # transposed copy first (conv critical path), then row-major
                xt = xpool.tile([128, 32 * 128], BF16, tag="xt")
                xtv = bass.AP(xt[:, :].tensor, xt[:, :].offset,
                              [[4096, 128], [128, 32], [1, 128]])
                nc.sync.dma_start_transpose(xtv, XS[r0:r0 + 128, 0:L])
                xsb = xpool.tile([128, XW], BF16, tag="xsb")
                nc.gpsimd.memset(xsb[:, 0:XOFF], 0.0)
                nc.gpsimd.memset(xsb[:, XOFF + L:XW], 0.0)
                for half in range(2):
                    c0 = 2048 * half
                    nc.sync.dma_start(xsb[:, XOFF + c0:XOFF + c0 + 2048],
                                      XS[r0:r0 + 128, c0:c0 + 2048])
                xsbs.append(xsb)
                xts.append(xt)
                if chunk == 0:
                    nc.sync.dma_start(cft[:, :], CF[:, :])

            def build_tables(chunk, d2_engine):
                xsb = xsbs[chunk]
                # d1t[:, i] = X[i] - X[i-1], i = 0..4096
                d1t = xpool.tile([128, L + 1], BF16, tag="d1t",
                                 name=f"d1t{chunk}")
                nc.vector.tensor_sub(d1t[:, 0:L + 1],
                                     xsb[:, XOFF:XOFF + L + 1],
                                     xsb[:, XOFF - 1:XOFF + L])
                # d2t[:, j] = D1[j] - D1[j-1] = d1t[j+1] - d1t[j]
                d2t = xpool.tile([128, L], BF16, tag="d2t",
                                 name=f"d2t{chunk}")
                d2_engine.tensor_sub(d2t[:, 0:L], d1t[:, 1:L + 1],
                                     d1t[:, 0:L])
                d1ts.append(d1t)
                d2ts.append(d2t)

            # chunk-0 tables early (d2 on GPSIMD: fills its startup hole);
            # chunk-1 tables are emitted mid-stream below (d2 on DVE)
            build_tables(0, nc.gpsimd)

            # ---- phase 2: conv -> decode -> interp, pipelined ----
            def decode_interp(chunk, pair, offpt, lo, hi, alb, beb, wb):
                """Decode patches [lo,hi) of this pair, build w slabs, interp,
                and DMA the outputs."""
                r0 = chunk * 128
                xsb, d1t, d2t = xsbs[chunk], d1ts[chunk], d2ts[chunk]
                E = nc.vector
                pn = hi - lo
                p0 = 256 * pair + lo          # global patch base (per chunk)
                dxv = _ap(offpt[:, :], 2 * lo, [[2, pn]])
                o1v = _ap(offpt[:, :], 2 * lo + 1, [[2, pn]])
                dsb = wpool.tile([128, 256], F32, tag="dsb", bufs=2)
                E.tensor_scalar(dsb[:, 0:pn], o1v, c_ds, 0.0, OP.add, OP.max)
                anb = wpool.tile([128, 256], F32, tag="anb", bufs=2)
                E.scalar_tensor_tensor(anb[:, 0:pn], dxv, b20,
                                       cview("AREP", col0=p0, ncols=pn),
                                       OP.add, OP.add)
                lob = wpool.tile([128, 256], F32, tag="lob", bufs=2)
                E.scalar_tensor_tensor(lob[:, 0:pn], anb[:, 0:pn], 0.0,
                                       dsb[:, 0:pn], OP.add, OP.subtract)
                hib = wpool.tile([128, 256], F32, tag="hib", bufs=2)
                E.scalar_tensor_tensor(hib[:, 0:pn], anb[:, 0:pn], 0.0,
                                       dsb[:, 0:pn], OP.add, OP.add)
                E.tensor_scalar(lob[:, 0:pn], lob[:, 0:pn], 0.0, float(L - 1),
                                OP.max, OP.min)
                E.tensor_scalar(hib[:, 0:pn], hib[:, 0:pn], 0.0, float(L - 1),
                                OP.max, OP.min)
                E.scalar_tensor_tensor(alb[:, lo:hi], lob[:, 0:pn], 0.0,
                                       cview("P8N", col0=p0, ncols=pn),
                                       OP.add, OP.add)
                E.scalar_tensor_tensor(beb[:, lo:hi], hib[:, 0:pn], 0.0,
                                       lob[:, 0:pn], OP.add, OP.subtract)
                E.tensor_scalar(beb[:, lo:hi], beb[:, lo:hi], 1.0 / 15.0,
                                -1.0, OP.mult, OP.add)

                # w slabs: w[:, 16p+s] = beta[p]*s + alpha[p]
                for s in range(PS):
                    wsl = _ap(wb[:, :], PS * lo + s, [[PS, pn]])
                    nc.vector.scalar_tensor_tensor(
                        wsl, beb[:, lo:hi], float(s), alb[:, lo:hi],
                        OP.mult, OP.add)

                # interp in sub-units of <=128 patches (clip to PC)
                for ulo in range(lo, hi, 128):
                    uhi = min(ulo + 128, hi)
                    p0i = 256 * pair + ulo
                    pbn = min(uhi - ulo, PC - p0i)
                    n = PS * pbn
                    wv = _ap(wb[:, :], PS * ulo, [[PS, pbn], [1, PS]])
                    x1v = _ap(xsb[:, :], XOFF + 8 * p0i, [[8, pbn], [1, PS]])
                    d1mv = _ap(d1t[:, :], 8 * p0i, [[8, pbn], [1, PS]])
                    d2v = _ap(d2t[:, :], 8 * p0i, [[8, pbn], [1, PS]])

                    rb = wpool.tile([128, 2048], BF16, tag="rb", bufs=2,
                                    name=f"rb{chunk}_{pair}_{ulo}")
                    rbv = _ap(rb[:, :], 0, [[PS, pbn], [1, PS]])
                    nc.vector.tensor_scalar_max(rbv, wv, 0.0)
                    t1 = wpool.tile([128, 2048], BF16, tag="t1", bufs=2,
                                    name=f"t1{chunk}_{pair}_{ulo}")
                    t1v = _ap(t1[:, :], 0, [[PS, pbn], [1, PS]])
                    nc.vector.tensor_mul(t1v, wv, d1mv)
                    # T2 = relu(w)*D2 (in-place into rb)
                    nc.gpsimd.tensor_mul(rbv, rbv, d2v)
                    # s1 = T1 + X[j]
                    nc.vector.tensor_add(t1v, t1v, x1v)
                    # final add in the DMA engines: OUT = s1, then OUT += T2
                    # (both on the gpsimd SWDGE queue => ordered)
                    oap = bass.AP(OUT[:].tensor, r0 * PC * PS + p0i * PS,
                                  [[PC * PS, 128], [1, n]])
                    nc.gpsimd.dma_start(oap, t1[:, 0:n])
                    nc.gpsimd.dma_start(oap, rb[:, 0:n], accum_op=OP.add)

            for chunk in range(NCHUNK):
                if chunk == 1:
                    build_tables(1, nc.vector)
                xt = xts[chunk]
                offpts = {}
                albs = {}
                # conv units of TBLK pairs; decode sub-units as their patch
                # ranges complete
                tstarts = list(range(0, NT, TBLK))
                done_subs = set()
                for t0 in tstarts:
                    tn = min(TBLK, NT - t0)
                    pt = ppool.tile([128, TBLK * 128], F32, tag="pt", bufs=2)
                    for q in range(tn):
                        t = t0 + q
                        blkA, rho = divmod(16 * t, 128)
                        dst = pt[:, 128 * q:128 * (q + 1)]
                        if rho <= 96:
                            nc.tensor.matmul(
                                dst, cview(f"W1R{rho}"),
                                xt[:, 128 * blkA:128 * (blkA + 1)],
                                start=True, stop=True)
                        elif t == NT - 1:
                            # patch 511 (discarded) needs block 32; skip
                            nc.tensor.matmul(
                                dst, cview("W1SA", p0=64, pn=64),
                                xt[64:128, 128 * blkA:128 * (blkA + 1)],
                                start=True, stop=True)
                        else:
                            nc.tensor.matmul(
                                dst, cview("W1SA", p0=64, pn=64),
                                xt[64:128, 128 * blkA:128 * (blkA + 1)],
                                start=True, stop=False)
                            nc.tensor.matmul(
                                dst, cview("W1SB", p0=0, pn=8),
                                xt[0:8, 128 * (blkA + 1):128 * (blkA + 2)],
                                start=False, stop=True)
                    hsb = wpool.tile([128, TBLK * 128], BF16, tag="hsb",
                                     bufs=3)
                    nc.scalar.activation(hsb[:, 0:128 * tn], pt[:, 0:128 * tn],
                                         AF.Gelu, bias=cview("B1P"),
                                         scale=1.0)
                    for q in range(tn):
                        t = t0 + q
                        pair = t // 128
                        if pair not in offpts:
                            offpts[pair] = ppool.tile(
                                [128, 512], F32, tag="offpt", bufs=2,
                                name=f"offpt{chunk}_{pair}")
                            alb = wpool.tile([128, 256], F32, tag="alb",
                                             bufs=2, name=f"alb{chunk}{pair}")
                            beb = wpool.tile([128, 256], F32, tag="beb",
                                             bufs=2, name=f"beb{chunk}{pair}")
                            wbt = wpool.tile([128, 4096], BF16, tag="wbt",
                                             bufs=2, name=f"wbt{chunk}{pair}")
                            albs[pair] = (alb, beb, wbt)
                        col = 4 * (t - 128 * pair)
                        nc.tensor.matmul(
                            offpts[pair][:, col:col + 4],
                            hsb[:, 128 * q:128 * (q + 1)],
                            cview("W2P"),
                            start=True, stop=True)

                    # emit any decode sub-units now complete
                    t_done = t0 + tn          # pairs fully conv'd below this
                    for pair in (0, 1):
                        subs = SUBS[(chunk, pair)]
                        for si in range(len(subs) - 1):
                            key = (pair, si)
                            if key in done_subs:
                                continue
                            # need conv2 for patches < 256*pair + subs[si+1],
                            # i.e. t < 128*pair + subs[si+1]/2
                            if 2 * t_done >= 256 * pair + subs[si + 1]:
                                alb, beb, wbt = albs[pair]
                                decode_interp(chunk, pair, offpts[pair],
                                              subs[si], subs[si + 1],
                                              alb, beb, wbt)
                                done_subs.add(key)
    nc.finalize()
    return nc


def kernel(X, W1, b1, W2, b2):
    X = np.ascontiguousarray(np.asarray(X, np.float32))
    tens, scal = _consts(W1, b1, W2, b2)
    key = tuple(sorted(scal.items()))
    if _CACHE.get("key") != key:
        _CACHE["nc"] = build(scal)
        _CACHE["key"] = key
    nc = _CACHE["nc"]

    Xb = X.astype(NPBF)
    in_maps = []
    for i in range(NCORES):
        m = {"XS": Xb[BPC * i:BPC * (i + 1)].reshape(ROWS, L)}
        m.update(tens)
        in_maps.append(m)

    res = run_bass_kernel_spmd(nc, in_maps, core_ids=list(range(NCORES)))
    out = np.concatenate(
        [np.asarray(res.results[i]["OUT"]).astype(np.float32)
         .reshape(BPC, C, PC, PS) for i in range(NCORES)], axis=0)
    return out


# revision 31
# speedup vs baseline: 1.0207x; 1.0207x over previous
"""Trainium2 Bass kernel for nn_DepatchSampling.

Strategy (hardcoded for B=32, C=64, L=4096, PS=16, STRIDE=8, PC=511, HID=64):

 - Pure data parallelism: batch dim (32) sharded over 8 cores, 4 batches each.
 - Per core, 256 (b,c) rows in 2 chunks of 128 rows (one row per partition).
 - Everything datapath-heavy runs in bf16 (validated: rel err ~2.8e-3 vs the
   2e-2 gate):
     * X is DMA'd in as bf16 twice: once row-major (xsb), once transposed
       into L-major 128-blocks via the DMA xbar (xt) — no PE transposes.
     * conv1 runs as bf16 matmuls (1 cyc/row), 12 patch-pairs per PSUM tile;
       gelu(+b1) on ACT; conv2 as tiny bf16 matmuls producing [row,
       (patch, j)] directly in PSUM.
 - Box decode (per patch sub-range, on DVE): ds = relu(o1+b2[1]+7.5);
   an = dx+b2[0]+8p+7.5; lo/hi = clip(an -/+ ds, 0, 4095);
   alpha = lo-8p, beta = (hi-lo)/15 - 1.
 - Sampling identity: with j = 8p+s and w = alpha + beta*s in [-1, 1):
       out = X[j] + w*D1[j-1] + relu(w)*D2[j],
   D1[i] = X[i]-X[i-1] (shifted), D2[j] = D1[j+1]-D1[j].  All accesses are
   static strided views.  w is built as 16 per-s strided slabs
   (w[:, s::16] = beta*s + alpha, one STT per s).  The packed bf16 ops
   (relu/mult/mult/add/add) are split between DVE (2x/4x modes) and GPSIMD
   for engine balance.  Decode/interp runs on progressively finer sub-units
   at the start and end of the schedule to shorten pipeline fill/drain.
 - Output stored bf16, upcast on host.
"""

import numpy as np
import ml_dtypes

import concourse.bass as bass
import concourse.bacc as bacc
import concourse.mybir as mybir
from concourse.tile import TileContext
from concourse.bass_utils import run_bass_kernel_spmd

F32 = mybir.dt.float32
BF16 = mybir.dt.bfloat16
AF = mybir.ActivationFunctionType
OP = mybir.AluOpType
NPBF = ml_dtypes.bfloat16

# Problem constants
B, C, L = 32, 64, 4096
PS, STRIDE, PC, HID = 16, 8, 511, 64
NCORES = 8
BPC = B // NCORES            # batches per core
ROWS = BPC * C               # 256 (b,c) rows per core
NCHUNK = 2                   # chunks of 128 rows
NT = 256                     # patch-pair index t per chunk: p = 2t, 2t+1
TBLK = 12                    # t per conv1 PSUM tile (conv unit)
XOFF = 8                     # X[j] lives at xsb[:, XOFF + j]
XW = XOFF + L + 8            # padded row width

# decode/interp sub-unit boundaries (in patches within a 256-patch pair),
# per (chunk, pair): finer at schedule start (early GPSIMD fill) and end
# (short drain).
SUBS = {
    (0, 0): [0, 64, 128, 256],
    (0, 1): [0, 128, 256],
    (1, 0): [0, 128, 256],
    (1, 1): [0, 128, 192, 256],
}

_CACHE = {}


def _consts(W1, b1, W2, b2):
    """Host-side packing of weights and constant tables."""
    W1 = np.asarray(W1, np.float32)
    b1 = np.asarray(b1, np.float32)
    W2 = np.asarray(W2, np.float32)
    b2 = np.asarray(b2, np.float32)

    # conv1 weight packs: pair t covers rows [16t, 16t+24) of the L axis;
    # within its 128-block the pair sits at row offset rho = 16*(t mod 8).
    # rho <= 96: single matmul with W1R{rho}; rho == 112: split into W1SA on
    # block A plus W1SB on block A+1, accumulated in PSUM.
    tens = {}
    for rho in range(0, 112, 16):
        full = np.zeros((128, 128), np.float32)
        full[rho:rho + 16, 0:64] = W1.T
        full[rho + 8:rho + 24, 64:128] = W1.T
        tens[f"W1R{rho}"] = full.astype(NPBF)
    w1sa = np.zeros((128, 128), np.float32)
    w1sa[112:128, 0:64] = W1.T
    w1sa[120:128, 64:128] = W1.T[0:8]       # odd patch s = 0..7
    tens["W1SA"] = w1sa.astype(NPBF)
    w1sb = np.zeros((128, 128), np.float32)
    w1sb[0:8, 64:128] = W1.T[8:16]          # odd patch s = 8..15
    tens["W1SB"] = w1sb.astype(NPBF)

    w2p = np.zeros((128, 4), np.float32)
    w2p[0:64, 0] = W2[0]
    w2p[0:64, 1] = W2[1]
    w2p[64:128, 2] = W2[0]
    w2p[64:128, 3] = W2[1]
    tens["W2P"] = w2p.astype(NPBF)
    tens["B1P"] = np.concatenate([b1, b1]).reshape(128, 1).astype(np.float32)

    p = np.arange(512, dtype=np.float32)
    tens["AREP"] = np.broadcast_to(8 * p + np.float32(7.5), (128, 512)).copy()
    tens["P8N"] = np.broadcast_to(-8 * p, (128, 512)).copy()

    # batch all constants into one bf16 + one f32 DMA (HWDGE dispatch is
    # serial at ~650ns each — 15 separate loads would stall startup)
    cb = np.concatenate([tens[k] for k in BF_ORDER], axis=1)
    cf = np.concatenate([tens[k].astype(np.float32) for k in F32_ORDER],
                        axis=1)
    packed = {"CB": np.ascontiguousarray(cb.astype(NPBF)),
              "CF": np.ascontiguousarray(cf.astype(np.float32))}

    scal = {
        "c_ds": float(np.float32(b2[1]) + np.float32(7.5)),
        "b20": float(np.float32(b2[0])),
    }
    return packed, scal


BF_ORDER = ([f"W1R{rho}" for rho in range(0, 112, 16)] +
            ["W1SA", "W1SB", "W2P"])
BF_COLS = [128] * 9 + [4]
F32_ORDER = ["B1P", "AREP", "P8N"]
F32_COLS = [1, 512, 512]


def _ap(tile_ap, col_off, dims):
    """Custom strided view of a 2D [128, F] tile: dims = [[step, count], ...]
    appended after the partition dim."""
    pstep = tile_ap.ap[0][0]
    npart = tile_ap.ap[0][1]
    return bass.AP(tile_ap.tensor, tile_ap.offset + col_off,
                   [[pstep, npart]] + [list(d) for d in dims])


def build(scal):
    nc = bacc.Bacc("TRN2", target_bir_lowering=False, debug=False)

    XS = nc.dram_tensor("XS", [ROWS, L], BF16, kind="ExternalInput")
    OUT = nc.dram_tensor("OUT", [ROWS, PC * PS], BF16, kind="ExternalOutput")
    CB = nc.dram_tensor("CB", [128, sum(BF_COLS)], BF16, kind="ExternalInput")
    CF = nc.dram_tensor("CF", [128, sum(F32_COLS)], F32, kind="ExternalInput")

    c_ds, b20 = scal["c_ds"], scal["b20"]

    with TileContext(nc) as tc:
        with tc.tile_pool(name="consts", bufs=1) as cpool, \
             tc.tile_pool(name="xbig", bufs=2) as xpool, \
             tc.tile_pool(name="work", bufs=2) as wpool, \
             tc.tile_pool(name="psum", bufs=2, space="PSUM") as ppool:

            xsbs, xts, d1ts, d2ts = [], [], [], []
            # ---- phase 1a: weights first, then chunk-0 X (HWDGE serial) ----
            cbt = cpool.tile([128, sum(BF_COLS)], BF16, tag="c_CB")
            nc.sync.dma_start(cbt[:, :], CB[:, :])
            cft = cpool.tile([128, sum(F32_COLS)], F32, tag="c_CF")
            ccol = {}
            c0 = 0
            for k, w in zip(BF_ORDER, BF_COLS):
                ccol[k] = (cbt, c0, w)
                c0 += w
            c0 = 0
            for k, w in zip(F32_ORDER, F32_COLS):
                ccol[k] = (cft, c0, w)
                c0 += w

            def cview(k, p0=0, pn=128, col0=0, ncols=None):
                tile, base, w = ccol[k]
                ap = tile[:, :]
                pstep = ap.ap[0][0]
                return bass.AP(ap.tensor,
                               ap.offset + p0 * pstep + base + col0,
                               [[pstep, pn],
                                [1, ncols if ncols is not None else w]])

            for chunk in range(NCHUNK):
                r0 = chunk * 128
                # transposed copy first (conv critical path), then row-major
                xt = xpool.tile([128, 32 * 128], BF16, tag="xt")
                xtv = bass.AP(xt[:, :].tensor, xt[:, :].offset,
                              [[4096, 128], [128, 32], [1, 128]])
                nc.sync.dma_start_transpose(xtv, XS[r0:r0 + 128, 0:L])
                xsb = xpool.tile([128, XW], BF16, tag="xsb")
                nc.gpsimd.memset(xsb[:, 0:XOFF], 0.0)
                nc.gpsimd.memset(xsb[:, XOFF + L:XW], 0.0)
                for half in range(2):
                    c0 = 2048 * half
                    nc.sync.dma_start(xsb[:, XOFF + c0:XOFF + c0 + 2048],
                                      XS[r0:r0 + 128, c0:c0 + 2048])
                xsbs.append(xsb)
                xts.append(xt)
                if chunk == 0:
                    nc.sync.dma_start(cft[:, :], CF[:, :])

            def build_tables(chunk, d2_engine):
                xsb = xsbs[chunk]
                # d1t[:, i] = X[i] - X[i-1], i = 0..4096
                d1t = xpool.tile([128, L + 1], BF16, tag="d1t",
                                 name=f"d1t{chunk}")
                nc.vector.tensor_sub(d1t[:, 0:L + 1],
                                     xsb[:, XOFF:XOFF + L + 1],
                                     xsb[:, XOFF - 1:XOFF + L])
                # d2t[:, j] = D1[j] - D1[j-1] = d1t[j+1] - d1t[j]
                d2t = xpool.tile([128, L], BF16, tag="d2t",
                                 name=f"d2t{chunk}")
                d2_engine.tensor_sub(d2t[:, 0:L], d1t[:, 1:L + 1],
                                     d1t[:, 0:L])
                d1ts.append(d1t)
                d2ts.append(d2t)

            # chunk-0 tables early (d2 on GPSIMD: fills its startup hole);
            # chunk-1 tables are emitted mid-stream below (d2 on DVE)
            build_tables(0, nc.gpsimd)

            # ---- phase 2: conv -> decode -> interp, pipelined ----
            def decode_interp(chunk, pair, offpt, lo, hi, alb, beb, wb):
                """Decode patches [lo,hi) of this pair, build w slabs, interp,
                and DMA the outputs."""
                r0 = chunk * 128
                xsb, d1t, d2t = xsbs[chunk], d1ts[chunk], d2ts[chunk]
                E = nc.vector
                pn = hi - lo
                p0 = 256 * pair + lo          # global patch base (per chunk)
                dxv = _ap(offpt[:, :], 2 * lo, [[2, pn]])
                o1v = _ap(offpt[:, :], 2 * lo + 1, [[2, pn]])
                dsb = wpool.tile([128, 256], F32, tag="dsb", bufs=2)
                E.tensor_scalar(dsb[:, 0:pn], o1v, c_ds, 0.0, OP.add, OP.max)
                anb = wpool.tile([128, 256], F32, tag="anb", bufs=2)
                E.scalar_tensor_tensor(anb[:, 0:pn], dxv, b20,
                                       cview("AREP", col0=p0, ncols=pn),
                                       OP.add, OP.add)
                lob = wpool.tile([128, 256], F32, tag="lob", bufs=2)
                E.scalar_tensor_tensor(lob[:, 0:pn], anb[:, 0:pn], 0.0,
                                       dsb[:, 0:pn], OP.add, OP.subtract)
                hib = wpool.tile([128, 256], F32, tag="hib", bufs=2)
                E.scalar_tensor_tensor(hib[:, 0:pn], anb[:, 0:pn], 0.0,
                                       dsb[:, 0:pn], OP.add, OP.add)
                E.tensor_scalar(lob[:, 0:pn], lob[:, 0:pn], 0.0, float(L - 1),
                                OP.max, OP.min)
                E.tensor_scalar(hib[:, 0:pn], hib[:, 0:pn], 0.0, float(L - 1),
                                OP.max, OP.min)
                E.scalar_tensor_tensor(alb[:, lo:hi], lob[:, 0:pn], 0.0,
                                       cview("P8N", col0=p0, ncols=pn),
                                       OP.add, OP.add)
                E.scalar_tensor_tensor(beb[:, lo:hi], hib[:, 0:pn], 0.0,
                                       lob[:, 0:pn], OP.add, OP.subtract)
                E.tensor_scalar(beb[:, lo:hi], beb[:, lo:hi], 1.0 / 15.0,
                                -1.0, OP.mult, OP.add)

                # w slabs: w[:, 16p+s] = beta[p]*s + alpha[p]
                for s in range(PS):
                    wsl = _ap(wb[:, :], PS * lo + s, [[PS, pn]])
                    nc.vector.scalar_tensor_tensor(
                        wsl, beb[:, lo:hi], float(s), alb[:, lo:hi],
                        OP.mult, OP.add)

                # interp in sub-units of <=128 patches (clip to PC)
                for ulo in range(lo, hi, 128):
                    uhi = min(ulo + 128, hi)
                    p0i = 256 * pair + ulo
                    pbn = min(uhi - ulo, PC - p0i)
                    n = PS * pbn
                    wv = _ap(wb[:, :], PS * ulo, [[PS, pbn], [1, PS]])
                    x1v = _ap(xsb[:, :], XOFF + 8 * p0i, [[8, pbn], [1, PS]])
                    d1mv = _ap(d1t[:, :], 8 * p0i, [[8, pbn], [1, PS]])
                    d2v = _ap(d2t[:, :], 8 * p0i, [[8, pbn], [1, PS]])

                    rb = wpool.tile([128, 2048], BF16, tag="rb", bufs=2,
                                    name=f"rb{chunk}_{pair}_{ulo}")
                    rbv = _ap(rb[:, :], 0, [[PS, pbn], [1, PS]])
                    nc.vector.tensor_scalar_max(rbv, wv, 0.0)
                    t1 = wpool.tile([128, 2048], BF16, tag="t1", bufs=2,
                                    name=f"t1{chunk}_{pair}_{ulo}")
                    t1v = _ap(t1[:, :], 0, [[PS, pbn], [1, PS]])
                    nc.vector.tensor_mul(t1v, wv, d1mv)
                    # T2 = relu(w)*D2 (in-place into rb)
                    nc.gpsimd.tensor_mul(rbv, rbv, d2v)
                    # s1 = T1 + X[j]
                    nc.vector.tensor_add(t1v, t1v, x1v)
                    # final add in the DMA engines: OUT = s1, then OUT += T2
                    # (both on the gpsimd SWDGE queue => ordered)
                    oap = bass.AP(OUT[:].tensor, r0 * PC * PS + p0i * PS,
                                  [[PC * PS, 128], [1, n]])
                    nc.gpsimd.dma_start(oap, t1[:, 0:n])
                    nc.gpsimd.dma_start(oap, rb[:, 0:n], accum_op=OP.add)

            for chunk in range(NCHUNK):
                if chunk == 1:
                    build_tables(1, nc.vector)
                xt = xts[chunk]
                offpts = {}
                albs = {}
                # conv units of TBLK pairs; decode sub-units as their patch
                # ranges complete
                tstarts = list(range(0, NT, TBLK))
                done_subs = set()
                for t0 in tstarts:
                    tn = min(TBLK, NT - t0)
                    pt = ppool.tile([128, TBLK * 128], F32, tag="pt", bufs=2)
                    for q in range(tn):
                        t = t0 + q
                        blkA, rho = divmod(16 * t, 128)
                        dst = pt[:, 128 * q:128 * (q + 1)]
                        if rho <= 96:
                            nc.tensor.matmul(
                                dst, cview(f"W1R{rho}"),
                                xt[:, 128 * blkA:128 * (blkA + 1)],
                                start=True, stop=True)
                        elif t == NT - 1:
                            # patch 511 (discarded) needs block 32; skip
                            nc.tensor.matmul(
                                dst, cview("W1SA", p0=64, pn=64),
                                xt[64:128, 128 * blkA:128 * (blkA + 1)],
                                start=True, stop=True)
                        else:
                            nc.tensor.matmul(
                                dst, cview("W1SA", p0=64, pn=64),
                                xt[64:128, 128 * blkA:128 * (blkA + 1)],
                                start=True, stop=False)
                            nc.tensor.matmul(
                                dst, cview("W1SB", p0=0, pn=8),
                                xt[0:8, 128 * (blkA + 1):128 * (blkA + 2)],
                                start=False, stop=True)
                    hsb = wpool.tile([128, TBLK * 128], BF16, tag="hsb",
                                     bufs=3)
                    nc.scalar.activation(hsb[:, 0:128 * tn], pt[:, 0:128 * tn],
                                         AF.Gelu, bias=cview("B1P"),
                                         scale=1.0)
                    for q in range(tn):
                        t = t0 + q
                        pair = t // 128
                        if pair not in offpts:
                            offpts[pair] = ppool.tile(
                                [128, 512], F32, tag="offpt", bufs=2,
                                name=f"offpt{chunk}_{pair}")
                            alb = wpool.tile([128, 256], F32, tag="alb",
                                             bufs=2, name=f"alb{chunk}{pair}")
                            beb = wpool.tile([128, 256], F32, tag="beb",
                                             bufs=2, name=f"beb{chunk}{pair}")
                            wbt = wpool.tile([128, 4096], BF16, tag="wbt",
                                             bufs=2, name=f"wbt{chunk}{pair}")
                            albs[pair] = (alb, beb, wbt)
                        col = 4 * (t - 128 * pair)
                        nc.tensor.matmul(
                            offpts[pair][:, col:col + 4],
                            hsb[:, 128 * q:128 * (q + 1)],
                            cview("W2P"),
                            start=True, stop=True)

                    # emit any decode sub-units now complete
                    t_done = t0 + tn          # pairs fully conv'd below this
                    for pair in (0, 1):
                        subs = SUBS[(chunk, pair)]
                        for si in range(len(subs) - 1):
                            key = (pair, si)
                            if key in done_subs:
                                continue
                            # need conv2 for patches < 256*pair + subs[si+1],
                            # i.e. t < 128*pair + subs[si+1]/2
                            if 2 * t_done >= 256 * pair + subs[si + 1]:
                                alb, beb, wbt = albs[pair]
                                decode_interp(chunk, pair, offpts[pair],
                                              subs[si], subs[si + 1],
                                              alb, beb, wbt)
                                done_subs.add(key)
    nc.finalize()
    return nc


def kernel(X, W1, b1, W2, b2):
    X = np.ascontiguousarray(np.asarray(X, np.float32))
    tens, scal = _consts(W1, b1, W2, b2)
    key = tuple(sorted(scal.items()))
    if _CACHE.get("key") != key:
        _CACHE["nc"] = build(scal)
        _CACHE["key"] = key
    nc = _CACHE["nc"]

    Xb = X.astype(NPBF)
    in_maps = []
    for i in range(NCORES):
        m = {"XS": Xb[BPC * i:BPC * (i + 1)].reshape(ROWS, L)}
        m.update(tens)
        in_maps.append(m)

    res = run_bass_kernel_spmd(nc, in_maps, core_ids=list(range(NCORES)))
    out = np.concatenate(
        [np.asarray(res.results[i]["OUT"]).astype(np.float32)
         .reshape(BPC, C, PC, PS) for i in range(NCORES)], axis=0)
    return out


# revision 37
# speedup vs baseline: 1.0324x; 1.0115x over previous
"""Trainium2 Bass kernel for nn_DepatchSampling.

Strategy (hardcoded for B=32, C=64, L=4096, PS=16, STRIDE=8, PC=511, HID=64):

 - Pure data parallelism: batch dim (32) sharded over 8 cores, 4 batches each.
 - Per core, 256 (b,c) rows in 2 chunks of 128 rows (one row per partition).
 - Everything datapath-heavy runs in bf16 (validated: rel err ~2.8e-3 vs the
   2e-2 gate):
     * X is DMA'd in as bf16 twice: once row-major (xsb), once transposed
       into L-major 128-blocks via the DMA xbar (xt) — no PE transposes.
     * conv1 runs as bf16 matmuls (1 cyc/row), 12 patch-pairs per PSUM tile;
       gelu(+b1) on ACT; conv2 as tiny bf16 matmuls producing [row,
       (patch, j)] directly in PSUM.
 - Box decode (per patch sub-range, on DVE): ds = relu(o1+b2[1]+7.5);
   an = dx+b2[0]+8p+7.5; lo/hi = clip(an -/+ ds, 0, 4095);
   alpha = lo-8p, beta = (hi-lo)/15 - 1.
 - Sampling identity: with j = 8p+s and w = alpha + beta*s in [-1, 1):
       out = X[j] + w*D1[j-1] + relu(w)*D2[j],
   D1[i] = X[i]-X[i-1] (shifted), D2[j] = D1[j+1]-D1[j].  All accesses are
   static strided views.  w is built as 16 per-s strided slabs
   (w[:, s::16] = beta*s + alpha, one STT per s).  The packed bf16 ops
   (relu/mult/mult/add/add) are split between DVE (2x/4x modes) and GPSIMD
   for engine balance.  Decode/interp runs on progressively finer sub-units
   at the start and end of the schedule to shorten pipeline fill/drain.
 - Output stored bf16, upcast on host.
"""

import numpy as np
import ml_dtypes

import concourse.bass as bass
import concourse.bacc as bacc
import concourse.mybir as mybir
from concourse.tile import TileContext
from concourse.bass_utils import run_bass_kernel_spmd

F32 = mybir.dt.float32
BF16 = mybir.dt.bfloat16
AF = mybir.ActivationFunctionType
OP = mybir.AluOpType
NPBF = ml_dtypes.bfloat16

# Problem constants
B, C, L = 32, 64, 4096
PS, STRIDE, PC, HID = 16, 8, 511, 64
NCORES = 8
BPC = B // NCORES            # batches per core
ROWS = BPC * C               # 256 (b,c) rows per core
NCHUNK = 2                   # chunks of 128 rows
NT = 256                     # patch-pair index t per chunk: p = 2t, 2t+1
TBLK = 12                    # t per conv1 PSUM tile (conv unit)
XOFF = 8                     # X[j] lives at xsb[:, XOFF + j]
XW = XOFF + L + 8            # padded row width

# decode/interp sub-unit boundaries (in patches within a 256-patch pair),
# per (chunk, pair): finer at schedule start (early GPSIMD fill) and end
# (short drain).
SUBS = {
    (0, 0): [0, 64, 128, 256],
    (0, 1): [0, 128, 256],
    (1, 0): [0, 128, 256],
    (1, 1): [0, 128, 192, 256],
}

_CACHE = {}


def _consts(W1, b1, W2, b2):
    """Host-side packing of weights and constant tables."""
    W1 = np.asarray(W1, np.float32)
    b1 = np.asarray(b1, np.float32)
    W2 = np.asarray(W2, np.float32)
    b2 = np.asarray(b2, np.float32)

    # conv1 weight packs: pair t covers rows [16t, 16t+24) of the L axis;
    # within its 128-block the pair sits at row offset rho = 16*(t mod 8).
    # rho <= 96: single matmul with W1R{rho}; rho == 112: split into W1SA on
    # block A plus W1SB on block A+1, accumulated in PSUM.
    tens = {}
    for rho in range(0, 112, 16):
        full = np.zeros((128, 128), np.float32)
        full[rho:rho + 16, 0:64] = W1.T
        full[rho + 8:rho + 24, 64:128] = W1.T
        tens[f"W1R{rho}"] = full.astype(NPBF)
    w1sa = np.zeros((128, 128), np.float32)
    w1sa[112:128, 0:64] = W1.T
    w1sa[120:128, 64:128] = W1.T[0:8]       # odd patch s = 0..7
    tens["W1SA"] = w1sa.astype(NPBF)
    w1sb = np.zeros((128, 128), np.float32)
    w1sb[0:8, 64:128] = W1.T[8:16]          # odd patch s = 8..15
    tens["W1SB"] = w1sb.astype(NPBF)

    w2p = np.zeros((128, 4), np.float32)
    w2p[0:64, 0] = W2[0]
    w2p[0:64, 1] = W2[1]
    w2p[64:128, 2] = W2[0]
    w2p[64:128, 3] = W2[1]
    tens["W2P"] = w2p.astype(NPBF)
    tens["B1P"] = np.concatenate([b1, b1]).reshape(128, 1).astype(np.float32)

    p = np.arange(512, dtype=np.float32)
    tens["AREP"] = np.broadcast_to(8 * p + np.float32(7.5), (128, 512)).copy()
    tens["P8N"] = np.broadcast_to(-8 * p, (128, 512)).copy()

    # batch all constants into one bf16 + one f32 DMA (HWDGE dispatch is
    # serial at ~650ns each — 15 separate loads would stall startup)
    cb = np.concatenate([tens[k] for k in BF_ORDER], axis=1)
    cf = np.concatenate([tens[k].astype(np.float32) for k in F32_ORDER],
                        axis=1)
    packed = {"CB": np.ascontiguousarray(cb.astype(NPBF)),
              "CF": np.ascontiguousarray(cf.astype(np.float32))}

    scal = {
        "c_ds": float(np.float32(b2[1]) + np.float32(7.5)),
        "b20": float(np.float32(b2[0])),
    }
    return packed, scal


BF_ORDER = ([f"W1R{rho}" for rho in range(0, 112, 16)] +
            ["W1SA", "W1SB", "W2P"])
BF_COLS = [128] * 9 + [4]
F32_ORDER = ["B1P", "AREP", "P8N"]
F32_COLS = [1, 512, 512]


def _ap(tile_ap, col_off, dims):
    """Custom strided view of a 2D [128, F] tile: dims = [[step, count], ...]
    appended after the partition dim."""
    pstep = tile_ap.ap[0][0]
    npart = tile_ap.ap[0][1]
    return bass.AP(tile_ap.tensor, tile_ap.offset + col_off,
                   [[pstep, npart]] + [list(d) for d in dims])


def build(scal):
    nc = bacc.Bacc("TRN2", target_bir_lowering=False, debug=False)

    XS = nc.dram_tensor("XS", [ROWS, L], BF16, kind="ExternalInput")
    OUT = nc.dram_tensor("OUT", [ROWS, PC * PS], BF16, kind="ExternalOutput")
    CB = nc.dram_tensor("CB", [128, sum(BF_COLS)], BF16, kind="ExternalInput")
    CF = nc.dram_tensor("CF", [128, sum(F32_COLS)], F32, kind="ExternalInput")

    c_ds, b20 = scal["c_ds"], scal["b20"]

    with TileContext(nc) as tc:
        with tc.tile_pool(name="consts", bufs=1) as cpool, \
             tc.tile_pool(name="xbig", bufs=2) as xpool, \
             tc.tile_pool(name="work", bufs=2) as wpool, \
             tc.tile_pool(name="psum", bufs=2, space="PSUM") as ppool:

            xsbs, xts, d1ts, d2ts = [], [], [], []
            # ---- phase 1a: weights first, then chunk-0 X (HWDGE serial) ----
            cbt = cpool.tile([128, sum(BF_COLS)], BF16, tag="c_CB")
            nc.sync.dma_start(cbt[:, :], CB[:, :])
            cft = cpool.tile([128, sum(F32_COLS)], F32, tag="c_CF")
            ccol = {}
            c0 = 0
            for k, w in zip(BF_ORDER, BF_COLS):
                ccol[k] = (cbt, c0, w)
                c0 += w
            c0 = 0
            for k, w in zip(F32_ORDER, F32_COLS):
                ccol[k] = (cft, c0, w)
                c0 += w

            def cview(k, p0=0, pn=128, col0=0, ncols=None):
                tile, base, w = ccol[k]
                ap = tile[:, :]
                pstep = ap.ap[0][0]
                return bass.AP(ap.tensor,
                               ap.offset + p0 * pstep + base + col0,
                               [[pstep, pn],
                                [1, ncols if ncols is not None else w]])

            # pre-trigger the ACT Gelu table load off the critical path
            atl = cpool.tile([128, 4], F32, tag="atl")
            nc.scalar.activation(atl[:, 0:1], cbt[:, 0:1], AF.Gelu,
                                 bias=0.0, scale=1.0)

            for chunk in range(NCHUNK):
                r0 = chunk * 128
                # transposed copy first (conv critical path), then row-major
                xt = xpool.tile([128, 32 * 128], BF16, tag="xt")
                xtv = bass.AP(xt[:, :].tensor, xt[:, :].offset,
                              [[4096, 128], [128, 32], [1, 128]])
                nc.sync.dma_start_transpose(xtv, XS[r0:r0 + 128, 0:L])
                xsb = xpool.tile([128, XW], BF16, tag="xsb")
                nc.gpsimd.memset(xsb[:, 0:XOFF], 0.0)
                nc.gpsimd.memset(xsb[:, XOFF + L:XW], 0.0)
                for half in range(2):
                    c0 = 2048 * half
                    nc.sync.dma_start(xsb[:, XOFF + c0:XOFF + c0 + 2048],
                                      XS[r0:r0 + 128, c0:c0 + 2048])
                xsbs.append(xsb)
                xts.append(xt)
                if chunk == 0:
                    nc.sync.dma_start(cft[:, :], CF[:, :])

            def build_tables(chunk, d2_engine):
                xsb = xsbs[chunk]
                # d1t[:, i] = X[i] - X[i-1], i = 0..4096
                d1t = xpool.tile([128, L + 1], BF16, tag="d1t",
                                 name=f"d1t{chunk}")
                nc.vector.tensor_sub(d1t[:, 0:L + 1],
                                     xsb[:, XOFF:XOFF + L + 1],
                                     xsb[:, XOFF - 1:XOFF + L])
                # d2t[:, j] = D1[j] - D1[j-1] = d1t[j+1] - d1t[j]
                d2t = xpool.tile([128, L], BF16, tag="d2t",
                                 name=f"d2t{chunk}")
                d2_engine.tensor_sub(d2t[:, 0:L], d1t[:, 1:L + 1],
                                     d1t[:, 0:L])
                d1ts.append(d1t)
                d2ts.append(d2t)

            # both chunks' tables early: d2 on GPSIMD fills its startup hole
            build_tables(0, nc.gpsimd)
            build_tables(1, nc.gpsimd)

            # ---- phase 2: conv -> decode -> interp, pipelined ----
            def decode_interp(chunk, pair, offpt, lo, hi, alb, beb, wb):
                """Decode patches [lo,hi) of this pair, build w slabs, interp,
                and DMA the outputs."""
                r0 = chunk * 128
                xsb, d1t, d2t = xsbs[chunk], d1ts[chunk], d2ts[chunk]
                E = nc.vector
                pn = hi - lo
                p0 = 256 * pair + lo          # global patch base (per chunk)
                dxv = _ap(offpt[:, :], 2 * lo, [[2, pn]])
                o1v = _ap(offpt[:, :], 2 * lo + 1, [[2, pn]])
                dsb = wpool.tile([128, 256], F32, tag="dsb", bufs=2)
                E.tensor_scalar(dsb[:, 0:pn], o1v, c_ds, 0.0, OP.add, OP.max)
                anb = wpool.tile([128, 256], F32, tag="anb", bufs=2)
                E.scalar_tensor_tensor(anb[:, 0:pn], dxv, b20,
                                       cview("AREP", col0=p0, ncols=pn),
                                       OP.add, OP.add)
                lob = wpool.tile([128, 256], F32, tag="lob", bufs=2)
                E.scalar_tensor_tensor(lob[:, 0:pn], anb[:, 0:pn], 0.0,
                                       dsb[:, 0:pn], OP.add, OP.subtract)
                hib = wpool.tile([128, 256], F32, tag="hib", bufs=2)
                E.scalar_tensor_tensor(hib[:, 0:pn], anb[:, 0:pn], 0.0,
                                       dsb[:, 0:pn], OP.add, OP.add)
                E.tensor_scalar(lob[:, 0:pn], lob[:, 0:pn], 0.0, float(L - 1),
                                OP.max, OP.min)
                E.tensor_scalar(hib[:, 0:pn], hib[:, 0:pn], 0.0, float(L - 1),
                                OP.max, OP.min)
                E.scalar_tensor_tensor(alb[:, lo:hi], lob[:, 0:pn], 0.0,
                                       cview("P8N", col0=p0, ncols=pn),
                                       OP.add, OP.add)
                E.scalar_tensor_tensor(beb[:, lo:hi], hib[:, 0:pn], 0.0,
                                       lob[:, 0:pn], OP.add, OP.subtract)
                E.tensor_scalar(beb[:, lo:hi], beb[:, lo:hi], 1.0 / 15.0,
                                -1.0, OP.mult, OP.add)

                # w[:, 16p+s] = beta[p]*s + alpha[p], built by log-doubling:
                # w[0]=alpha; w[s..2s-1] = beta*s + w[0..s-1]
                w0 = _ap(wb[:, :], PS * lo, [[PS, pn]])
                nc.vector.tensor_copy(w0, _ap(alb[:, :], lo, [[1, pn]]))
                for sh in (1, 2, 4, 8):
                    wdst = _ap(wb[:, :], PS * lo + sh, [[PS, pn], [1, sh]])
                    wsrc = _ap(wb[:, :], PS * lo, [[PS, pn], [1, sh]])
                    bbc = _ap(beb[:, :], lo, [[1, pn], [0, sh]])
                    nc.vector.scalar_tensor_tensor(
                        wdst, bbc, float(sh), wsrc, OP.mult, OP.add)

                # interp in sub-units of <=128 patches (clip to PC)
                for ulo in range(lo, hi, 128):
                    uhi = min(ulo + 128, hi)
                    p0i = 256 * pair + ulo
                    pbn = min(uhi - ulo, PC - p0i)
                    n = PS * pbn
                    wv = _ap(wb[:, :], PS * ulo, [[PS, pbn], [1, PS]])
                    x1v = _ap(xsb[:, :], XOFF + 8 * p0i, [[8, pbn], [1, PS]])
                    d1mv = _ap(d1t[:, :], 8 * p0i, [[8, pbn], [1, PS]])
                    d2v = _ap(d2t[:, :], 8 * p0i, [[8, pbn], [1, PS]])

                    rb = wpool.tile([128, 2048], BF16, tag="rb", bufs=3,
                                    name=f"rb{chunk}_{pair}_{ulo}")
                    rbv = _ap(rb[:, :], 0, [[PS, pbn], [1, PS]])
                    nc.vector.tensor_scalar_max(rbv, wv, 0.0)
                    t1 = wpool.tile([128, 2048], BF16, tag="t1", bufs=3,
                                    name=f"t1{chunk}_{pair}_{ulo}")
                    t1v = _ap(t1[:, :], 0, [[PS, pbn], [1, PS]])
                    nc.vector.tensor_mul(t1v, wv, d1mv)
                    # T2 = relu(w)*D2 (in-place into rb); the final unit's
                    # runs on DVE to shorten the drain
                    last = (chunk == 1 and pair == 1 and uhi == 256)
                    (nc.vector if last else nc.gpsimd).tensor_mul(
                        rbv, rbv, d2v)
                    # s1 = T1 + X[j]
                    nc.vector.tensor_add(t1v, t1v, x1v)
                    # final add in the DMA engines: OUT = s1, then OUT += T2
                    # (both on the gpsimd SWDGE queue => ordered)
                    oap = bass.AP(OUT[:].tensor, r0 * PC * PS + p0i * PS,
                                  [[PC * PS, 128], [1, n]])
                    nc.gpsimd.dma_start(oap, t1[:, 0:n])
                    nc.gpsimd.dma_start(oap, rb[:, 0:n], accum_op=OP.add)

            for chunk in range(NCHUNK):
                xt = xts[chunk]
                offpts = {}
                albs = {}
                # conv units of TBLK pairs; decode sub-units as their patch
                # ranges complete
                tstarts = list(range(0, NT, TBLK))
                done_subs = set()
                for t0 in tstarts:
                    tn = min(TBLK, NT - t0)
                    pt = ppool.tile([128, TBLK * 128], F32, tag="pt", bufs=2)
                    for q in range(tn):
                        t = t0 + q
                        blkA, rho = divmod(16 * t, 128)
                        dst = pt[:, 128 * q:128 * (q + 1)]
                        if rho <= 96:
                            nc.tensor.matmul(
                                dst, cview(f"W1R{rho}"),
                                xt[:, 128 * blkA:128 * (blkA + 1)],
                                start=True, stop=True)
                        elif t == NT - 1:
                            # patch 511 (discarded) needs block 32; skip
                            nc.tensor.matmul(
                                dst, cview("W1SA", p0=64, pn=64),
                                xt[64:128, 128 * blkA:128 * (blkA + 1)],
                                start=True, stop=True)
                        else:
                            nc.tensor.matmul(
                                dst, cview("W1SA", p0=64, pn=64),
                                xt[64:128, 128 * blkA:128 * (blkA + 1)],
                                start=True, stop=False)
                            nc.tensor.matmul(
                                dst, cview("W1SB", p0=0, pn=8),
                                xt[0:8, 128 * (blkA + 1):128 * (blkA + 2)],
                                start=False, stop=True)
                    hsb = wpool.tile([128, TBLK * 128], BF16, tag="hsb",
                                     bufs=3)
                    nc.scalar.activation(hsb[:, 0:128 * tn], pt[:, 0:128 * tn],
                                         AF.Gelu, bias=cview("B1P"),
                                         scale=1.0)
                    for q in range(tn):
                        t = t0 + q
                        pair = t // 128
                        if pair not in offpts:
                            offpts[pair] = ppool.tile(
                                [128, 512], F32, tag="offpt", bufs=2,
                                name=f"offpt{chunk}_{pair}")
                            alb = wpool.tile([128, 256], F32, tag="alb",
                                             bufs=2, name=f"alb{chunk}{pair}")
                            beb = wpool.tile([128, 256], F32, tag="beb",
                                             bufs=2, name=f"beb{chunk}{pair}")
                            wbt = wpool.tile([128, 4096], BF16, tag="wbt",
                                             bufs=2, name=f"wbt{chunk}{pair}")
                            albs[pair] = (alb, beb, wbt)
                        col = 4 * (t - 128 * pair)
                        nc.tensor.matmul(
                            offpts[pair][:, col:col + 4],
                            hsb[:, 128 * q:128 * (q + 1)],
                            cview("W2P"),
                            start=True, stop=True)

                    # emit any decode sub-units now complete
                    t_done = t0 + tn          # pairs fully conv'd below this
                    for pair in (0, 1):
                        subs = SUBS[(chunk, pair)]
                        for si in range(len(subs) - 1):
                            key = (pair, si)
                            if key in done_subs:
                                continue
                            # need conv2 for patches < 256*pair + subs[si+1],
                            # i.e. t < 128*pair + subs[si+1]/2
                            if 2 * t_done >= 256 * pair + subs[si + 1]:
                                alb, beb, wbt = albs[pair]
                                decode_interp(chunk, pair, offpts[pair],
                                              subs[si], subs[si + 1],
                                              alb, beb, wbt)
                                done_subs.add(key)
    nc.finalize()
    return nc


def kernel(X, W1, b1, W2, b2):
    X = np.ascontiguousarray(np.asarray(X, np.float32))
    tens, scal = _consts(W1, b1, W2, b2)
    key = tuple(sorted(scal.items()))
    if _CACHE.get("key") != key:
        _CACHE["nc"] = build(scal)
        _CACHE["key"] = key
    nc = _CACHE["nc"]

    Xb = X.astype(NPBF)
    in_maps = []
    for i in range(NCORES):
        m = {"XS": Xb[BPC * i:BPC * (i + 1)].reshape(ROWS, L)}
        m.update(tens)
        in_maps.append(m)

    res = run_bass_kernel_spmd(nc, in_maps, core_ids=list(range(NCORES)))
    out = np.concatenate(
        [np.asarray(res.results[i]["OUT"]).astype(np.float32)
         .reshape(BPC, C, PC, PS) for i in range(NCORES)], axis=0)
    return out


# revision 46
# speedup vs baseline: 1.1747x; 1.1379x over previous
"""Trainium2 Bass kernel for nn_DepatchSampling.

Strategy (hardcoded for B=32, C=64, L=4096, PS=16, STRIDE=8, PC=511, HID=64):

 - Pure data parallelism: batch dim (32) sharded over 8 cores, 4 batches each.
 - Per core, 256 (b,c) rows in 2 chunks of 128 rows (one row per partition).
 - Everything datapath-heavy runs in bf16 (validated: rel err ~2.8e-3 vs the
   2e-2 gate):
     * X is DMA'd in as bf16 twice: once row-major (xsb), once transposed
       into L-major 128-blocks via the DMA xbar (xt) — no PE transposes.
     * conv1 runs as bf16 matmuls (1 cyc/row), 12 patch-pairs per PSUM tile;
       gelu(+b1) on ACT; conv2 as tiny bf16 matmuls producing [row,
       (patch, j)] directly in PSUM.
 - Box decode (per patch sub-range, on DVE): ds = relu(o1+b2[1]+7.5);
   an = dx+b2[0]+8p+7.5; lo/hi = clip(an -/+ ds, 0, 4095);
   alpha = lo-8p, beta = (hi-lo)/15 - 1.
 - Sampling identity: with j = 8p+s and w = alpha + beta*s in [-1, 1):
       out = X[j] + w*D1[j-1] + relu(w)*D2[j],
   D1[i] = X[i]-X[i-1] (shifted), D2[j] = D1[j+1]-D1[j].  All accesses are
   static strided views.  w is built as 16 per-s strided slabs
   (w[:, s::16] = beta*s + alpha, one STT per s).  The packed bf16 ops
   (relu/mult/mult/add/add) are split between DVE (2x/4x modes) and GPSIMD
   for engine balance.  Decode/interp runs on progressively finer sub-units
   at the start and end of the schedule to shorten pipeline fill/drain.
 - Output stored bf16, upcast on host.
"""

import numpy as np
import ml_dtypes

import concourse.bass as bass
import concourse.bacc as bacc
import concourse.mybir as mybir
from concourse.tile import TileContext
from concourse.bass_utils import run_bass_kernel_spmd

F32 = mybir.dt.float32
BF16 = mybir.dt.bfloat16
AF = mybir.ActivationFunctionType
OP = mybir.AluOpType
NPBF = ml_dtypes.bfloat16

# Problem constants
B, C, L = 32, 64, 4096
PS, STRIDE, PC, HID = 16, 8, 511, 64
NCORES = 8
BPC = B // NCORES            # batches per core
ROWS = BPC * C               # 256 (b,c) rows per core
NCHUNK = 2                   # chunks of 128 rows
NT = 256                     # patch-pair index t per chunk: p = 2t, 2t+1
TBLK = 12                    # t per conv1 PSUM tile (conv unit)
XOFF = 8                     # X[j] lives at xsb[:, XOFF + j]
XW = XOFF + L + 8            # padded row width

# decode/interp sub-unit boundaries (in patches within a 256-patch pair),
# per (chunk, pair): finer at schedule start (early GPSIMD fill) and end
# (short drain).
SUBS = {
    (0, 0): [0, 64, 128, 256],
    (0, 1): [0, 128, 256],
    (1, 0): [0, 128, 256],
    (1, 1): [0, 128, 192, 256],
}

_CACHE = {}


def _consts(W1, b1, W2, b2):
    """Host-side packing of weights and constant tables."""
    W1 = np.asarray(W1, np.float32)
    b1 = np.asarray(b1, np.float32)
    W2 = np.asarray(W2, np.float32)
    b2 = np.asarray(b2, np.float32)

    # conv1 weight packs: pair t covers rows [16t, 16t+24) of the L axis;
    # within its 128-block the pair sits at row offset rho = 16*(t mod 8).
    # rho <= 96: single matmul with W1R{rho}; rho == 112: split into W1SA on
    # block A plus W1SB on block A+1, accumulated in PSUM.
    tens = {}
    for rho in range(0, 112, 16):
        full = np.zeros((128, 128), np.float32)
        full[rho:rho + 16, 0:64] = W1.T
        full[rho + 8:rho + 24, 64:128] = W1.T
        tens[f"W1R{rho}"] = full.astype(NPBF)
    w1sa = np.zeros((128, 128), np.float32)
    w1sa[112:128, 0:64] = W1.T
    w1sa[120:128, 64:128] = W1.T[0:8]       # odd patch s = 0..7
    tens["W1SA"] = w1sa.astype(NPBF)
    w1sb = np.zeros((128, 128), np.float32)
    w1sb[0:8, 64:128] = W1.T[8:16]          # odd patch s = 8..15
    tens["W1SB"] = w1sb.astype(NPBF)

    w2p = np.zeros((128, 4), np.float32)
    w2p[0:64, 0] = W2[0]
    w2p[0:64, 1] = W2[1]
    w2p[64:128, 2] = W2[0]
    w2p[64:128, 3] = W2[1]
    tens["W2P"] = w2p.astype(NPBF)
    tens["B1P"] = np.concatenate([b1, b1]).reshape(128, 1).astype(np.float32)

    p = np.arange(512, dtype=np.float32)
    tens["AREP"] = np.broadcast_to(8 * p + np.float32(7.5), (128, 512)).copy()
    tens["P8N"] = np.broadcast_to(-8 * p, (128, 512)).copy()

    # batch all constants into one bf16 + one f32 DMA (HWDGE dispatch is
    # serial at ~650ns each — 15 separate loads would stall startup)
    cb = np.concatenate([tens[k] for k in BF_ORDER], axis=1)
    cf = np.concatenate([tens[k].astype(np.float32) for k in F32_ORDER],
                        axis=1)
    packed = {"CB": np.ascontiguousarray(cb.astype(NPBF)),
              "CF": np.ascontiguousarray(cf.astype(np.float32))}

    scal = {
        "c_ds": float(np.float32(b2[1]) + np.float32(7.5)),
        "b20": float(np.float32(b2[0])),
    }
    return packed, scal


BF_ORDER = ([f"W1R{rho}" for rho in range(0, 112, 16)] +
            ["W1SA", "W1SB", "W2P"])
BF_COLS = [128] * 9 + [4]
F32_ORDER = ["B1P", "AREP", "P8N"]
F32_COLS = [1, 512, 512]


def _ap(tile_ap, col_off, dims):
    """Custom strided view of a 2D [128, F] tile: dims = [[step, count], ...]
    appended after the partition dim."""
    pstep = tile_ap.ap[0][0]
    npart = tile_ap.ap[0][1]
    return bass.AP(tile_ap.tensor, tile_ap.offset + col_off,
                   [[pstep, npart]] + [list(d) for d in dims])


def build(scal):
    nc = bacc.Bacc("TRN2", target_bir_lowering=False, debug=False)

    XS = nc.dram_tensor("XS", [ROWS, L], BF16, kind="ExternalInput")
    OUT = nc.dram_tensor("OUT", [ROWS, PC * PS], BF16, kind="ExternalOutput")
    CB = nc.dram_tensor("CB", [128, sum(BF_COLS)], BF16, kind="ExternalInput")
    CF = nc.dram_tensor("CF", [128, sum(F32_COLS)], F32, kind="ExternalInput")

    c_ds, b20 = scal["c_ds"], scal["b20"]

    with TileContext(nc) as tc:
        with tc.tile_pool(name="consts", bufs=1) as cpool, \
             tc.tile_pool(name="xbig", bufs=2) as xpool, \
             tc.tile_pool(name="work", bufs=2) as wpool, \
             tc.tile_pool(name="psum", bufs=2, space="PSUM") as ppool:

            xsbs, xts, d1ts, d2ts = [], [], [], []
            # ---- phase 1a: weights first, then chunk-0 X (HWDGE serial) ----
            cbt = cpool.tile([128, sum(BF_COLS)], BF16, tag="c_CB")
            nc.sync.dma_start(cbt[:, :], CB[:, :])
            cft = cpool.tile([128, sum(F32_COLS)], F32, tag="c_CF")
            ccol = {}
            c0 = 0
            for k, w in zip(BF_ORDER, BF_COLS):
                ccol[k] = (cbt, c0, w)
                c0 += w
            c0 = 0
            for k, w in zip(F32_ORDER, F32_COLS):
                ccol[k] = (cft, c0, w)
                c0 += w

            def cview(k, p0=0, pn=128, col0=0, ncols=None):
                tile, base, w = ccol[k]
                ap = tile[:, :]
                pstep = ap.ap[0][0]
                return bass.AP(ap.tensor,
                               ap.offset + p0 * pstep + base + col0,
                               [[pstep, pn],
                                [1, ncols if ncols is not None else w]])

            # pre-trigger the ACT Gelu table load off the critical path
            atl = cpool.tile([128, 4], F32, tag="atl")
            nc.scalar.activation(atl[:, 0:1], cbt[:, 0:1], AF.Gelu,
                                 bias=0.0, scale=1.0)

            for chunk in range(NCHUNK):
                r0 = chunk * 128
                xsb = xpool.tile([128, XW], BF16, tag="xsb")
                nc.gpsimd.memset(xsb[:, 0:XOFF], 0.0)
                nc.gpsimd.memset(xsb[:, XOFF + L:XW], 0.0)
                for half in range(2):
                    c0 = 2048 * half
                    nc.sync.dma_start(xsb[:, XOFF + c0:XOFF + c0 + 2048],
                                      XS[r0:r0 + 128, c0:c0 + 2048])
                # transposed copy (L-major 128-blocks) via DMA xbar
                xt = xpool.tile([128, 32 * 128], BF16, tag="xt")
                xtv = bass.AP(xt[:, :].tensor, xt[:, :].offset,
                              [[4096, 128], [128, 32], [1, 128]])
                nc.sync.dma_start_transpose(xtv, XS[r0:r0 + 128, 0:L])
                xsbs.append(xsb)
                xts.append(xt)
                if chunk == 0:
                    nc.sync.dma_start(cft[:, :], CF[:, :])

            def build_tables(chunk, d2_engine):
                xsb = xsbs[chunk]
                # d1t[:, i] = X[i] - X[i-1], i = 0..4096
                d1t = xpool.tile([128, L + 1], BF16, tag="d1t",
                                 name=f"d1t{chunk}")
                nc.vector.tensor_sub(d1t[:, 0:L + 1],
                                     xsb[:, XOFF:XOFF + L + 1],
                                     xsb[:, XOFF - 1:XOFF + L])
                # d2t[:, j] = D1[j] - D1[j-1] = d1t[j+1] - d1t[j]
                d2t = xpool.tile([128, L], BF16, tag="d2t",
                                 name=f"d2t{chunk}")
                d2_engine.tensor_sub(d2t[:, 0:L], d1t[:, 1:L + 1],
                                     d1t[:, 0:L])
                d1ts.append(d1t)
                d2ts.append(d2t)

            # chunk-0 d2 on GPSIMD (fills its startup hole); chunk-1 on DVE
            build_tables(0, nc.gpsimd)
            build_tables(1, nc.vector)

            # ---- phase 2: conv -> decode -> interp, pipelined ----
            def decode_interp(chunk, pair, offpt, lo, hi, alb, beb, wb):
                """Decode patches [lo,hi) of this pair, build w slabs, interp,
                and DMA the outputs."""
                r0 = chunk * 128
                xsb, d1t, d2t = xsbs[chunk], d1ts[chunk], d2ts[chunk]
                E = nc.vector
                pn = hi - lo
                p0 = 256 * pair + lo          # global patch base (per chunk)
                dxv = _ap(offpt[:, :], 2 * lo, [[2, pn]])
                o1v = _ap(offpt[:, :], 2 * lo + 1, [[2, pn]])
                dsb = wpool.tile([128, 256], F32, tag="dsb", bufs=2)
                E.tensor_scalar(dsb[:, 0:pn], o1v, c_ds, 0.0, OP.add, OP.max)
                anb = wpool.tile([128, 256], F32, tag="anb", bufs=2)
                E.scalar_tensor_tensor(anb[:, 0:pn], dxv, b20,
                                       cview("AREP", col0=p0, ncols=pn),
                                       OP.add, OP.add)
                lob = wpool.tile([128, 256], F32, tag="lob", bufs=2)
                E.scalar_tensor_tensor(lob[:, 0:pn], anb[:, 0:pn], 0.0,
                                       dsb[:, 0:pn], OP.add, OP.subtract)
                hib = wpool.tile([128, 256], F32, tag="hib", bufs=2)
                E.scalar_tensor_tensor(hib[:, 0:pn], anb[:, 0:pn], 0.0,
                                       dsb[:, 0:pn], OP.add, OP.add)
                E.tensor_scalar(lob[:, 0:pn], lob[:, 0:pn], 0.0, float(L - 1),
                                OP.max, OP.min)
                E.tensor_scalar(hib[:, 0:pn], hib[:, 0:pn], 0.0, float(L - 1),
                                OP.max, OP.min)
                E.scalar_tensor_tensor(alb[:, lo:hi], lob[:, 0:pn], 0.0,
                                       cview("P8N", col0=p0, ncols=pn),
                                       OP.add, OP.add)
                E.scalar_tensor_tensor(beb[:, lo:hi], hib[:, 0:pn], 0.0,
                                       lob[:, 0:pn], OP.add, OP.subtract)
                E.tensor_scalar(beb[:, lo:hi], beb[:, lo:hi], 1.0 / 15.0,
                                -1.0, OP.mult, OP.add)

                # w[:, 16p+s] = beta[p]*s + alpha[p], built by log-doubling:
                # w[0]=alpha; w[s..2s-1] = beta*s + w[0..s-1]
                w0 = _ap(wb[:, :], PS * lo, [[PS, pn]])
                nc.vector.tensor_copy(w0, _ap(alb[:, :], lo, [[1, pn]]))
                for sh in (1, 2, 4, 8):
                    wdst = _ap(wb[:, :], PS * lo + sh, [[PS, pn], [1, sh]])
                    wsrc = _ap(wb[:, :], PS * lo, [[PS, pn], [1, sh]])
                    bbc = _ap(beb[:, :], lo, [[1, pn], [0, sh]])
                    nc.vector.scalar_tensor_tensor(
                        wdst, bbc, float(sh), wsrc, OP.mult, OP.add)

                # interp in sub-units of <=128 patches (clip to PC)
                for ulo in range(lo, hi, 128):
                    uhi = min(ulo + 128, hi)
                    p0i = 256 * pair + ulo
                    pbn = min(uhi - ulo, PC - p0i)
                    n = PS * pbn
                    wv = _ap(wb[:, :], PS * ulo, [[PS, pbn], [1, PS]])
                    x1v = _ap(xsb[:, :], XOFF + 8 * p0i, [[8, pbn], [1, PS]])
                    d1mv = _ap(d1t[:, :], 8 * p0i, [[8, pbn], [1, PS]])
                    d2v = _ap(d2t[:, :], 8 * p0i, [[8, pbn], [1, PS]])

                    rb = wpool.tile([128, 2048], BF16, tag="rb", bufs=3,
                                    name=f"rb{chunk}_{pair}_{ulo}")
                    rbv = _ap(rb[:, :], 0, [[PS, pbn], [1, PS]])
                    late = (chunk == 1 and (pair == 1 or ulo >= 128))
                    if late:
                        # ACT is idle during the drain phase
                        nc.scalar.activation(rbv, wv, AF.Relu, bias=0.0,
                                             scale=1.0)
                    else:
                        nc.vector.tensor_scalar_max(rbv, wv, 0.0)
                    t1 = wpool.tile([128, 2048], BF16, tag="t1", bufs=3,
                                    name=f"t1{chunk}_{pair}_{ulo}")
                    t1v = _ap(t1[:, :], 0, [[PS, pbn], [1, PS]])
                    nc.vector.tensor_mul(t1v, wv, d1mv)
                    # T2 = relu(w)*D2 (in-place into rb)
                    nc.gpsimd.tensor_mul(rbv, rbv, d2v)
                    # s1 = T1 + X[j]
                    nc.vector.tensor_add(t1v, t1v, x1v)
                    # final add in the DMA engines: OUT = s1, then OUT += T2
                    # (both on the gpsimd SWDGE queue => ordered)
                    oap = bass.AP(OUT[:].tensor, r0 * PC * PS + p0i * PS,
                                  [[PC * PS, 128], [1, n]])
                    nc.gpsimd.dma_start(oap, t1[:, 0:n])
                    nc.gpsimd.dma_start(oap, rb[:, 0:n], accum_op=OP.add)

            for chunk in range(NCHUNK):
                xt = xts[chunk]
                offpts = {}
                albs = {}
                # conv units of TBLK pairs; decode sub-units as their patch
                # ranges complete
                tstarts = list(range(0, NT, TBLK))
                done_subs = set()
                for t0 in tstarts:
                    tn = min(TBLK, NT - t0)
                    pt = ppool.tile([128, TBLK * 128], F32, tag="pt", bufs=2)
                    for q in range(tn):
                        t = t0 + q
                        blkA, rho = divmod(16 * t, 128)
                        dst = pt[:, 128 * q:128 * (q + 1)]
                        if rho <= 96:
                            nc.tensor.matmul(
                                dst, cview(f"W1R{rho}"),
                                xt[:, 128 * blkA:128 * (blkA + 1)],
                                start=True, stop=True)
                        elif t == NT - 1:
                            # patch 511 (discarded) needs block 32; skip
                            nc.tensor.matmul(
                                dst, cview("W1SA", p0=64, pn=64),
                                xt[64:128, 128 * blkA:128 * (blkA + 1)],
                                start=True, stop=True)
                        else:
                            nc.tensor.matmul(
                                dst, cview("W1SA", p0=64, pn=64),
                                xt[64:128, 128 * blkA:128 * (blkA + 1)],
                                start=True, stop=False)
                            nc.tensor.matmul(
                                dst, cview("W1SB", p0=0, pn=8),
                                xt[0:8, 128 * (blkA + 1):128 * (blkA + 2)],
                                start=False, stop=True)
                    hsb = wpool.tile([128, TBLK * 128], BF16, tag="hsb",
                                     bufs=4)
                    nc.scalar.activation(hsb[:, 0:128 * tn], pt[:, 0:128 * tn],
                                         AF.Gelu, bias=cview("B1P"),
                                         scale=1.0)
                    for q in range(tn):
                        t = t0 + q
                        pair = t // 128
                        if pair not in offpts:
                            offpts[pair] = ppool.tile(
                                [128, 512], F32, tag="offpt", bufs=2,
                                name=f"offpt{chunk}_{pair}")
                            alb = wpool.tile([128, 256], F32, tag="alb",
                                             bufs=2, name=f"alb{chunk}{pair}")
                            beb = wpool.tile([128, 256], F32, tag="beb",
                                             bufs=2, name=f"beb{chunk}{pair}")
                            wbt = wpool.tile([128, 4096], BF16, tag="wbt",
                                             bufs=2, name=f"wbt{chunk}{pair}")
                            albs[pair] = (alb, beb, wbt)
                        col = 4 * (t - 128 * pair)
                        nc.tensor.matmul(
                            offpts[pair][:, col:col + 4],
                            hsb[:, 128 * q:128 * (q + 1)],
                            cview("W2P"),
                            start=True, stop=True)

                    # emit any decode sub-units now complete
                    t_done = t0 + tn          # pairs fully conv'd below this
                    for pair in (0, 1):
                        subs = SUBS[(chunk, pair)]
                        for si in range(len(subs) - 1):
                            key = (pair, si)
                            if key in done_subs:
                                continue
                            # need conv2 for patches < 256*pair + subs[si+1],
                            # i.e. t < 128*pair + subs[si+1]/2
                            if 2 * t_done >= 256 * pair + subs[si + 1]:
                                alb, beb, wbt = albs[pair]
                                decode_interp(chunk, pair, offpts[pair],
                                              subs[si], subs[si + 1],
                                              alb, beb, wbt)
                                done_subs.add(key)
    nc.finalize()
    return nc


def kernel(X, W1, b1, W2, b2):
    X = np.ascontiguousarray(np.asarray(X, np.float32))
    tens, scal = _consts(W1, b1, W2, b2)
    key = tuple(sorted(scal.items()))
    if _CACHE.get("key") != key:
        _CACHE["nc"] = build(scal)
        _CACHE["key"] = key
    nc = _CACHE["nc"]

    Xb = X.astype(NPBF)
    in_maps = []
    for i in range(NCORES):
        m = {"XS": Xb[BPC * i:BPC * (i + 1)].reshape(ROWS, L)}
        m.update(tens)
        in_maps.append(m)

    res = run_bass_kernel_spmd(nc, in_maps, core_ids=list(range(NCORES)))
    out = np.concatenate(
        [np.asarray(res.results[i]["OUT"]).astype(np.float32)
         .reshape(BPC, C, PC, PS) for i in range(NCORES)], axis=0)
    return out


# revision 57
# speedup vs baseline: 1.2277x; 1.0451x over previous
"""Trainium2 Bass kernel for nn_DepatchSampling.

Strategy (hardcoded for B=32, C=64, L=4096, PS=16, STRIDE=8, PC=511, HID=64):

 - Pure data parallelism: batch dim (32) sharded over 8 cores, 4 batches each.
 - Per core, 256 (b,c) rows in 2 chunks of 128 rows (one row per partition).
 - Everything datapath-heavy runs in bf16 (validated: rel err ~2.8e-3 vs the
   2e-2 gate):
     * X is DMA'd in as bf16 twice: once row-major (xsb), once transposed
       into L-major 128-blocks via the DMA xbar (xt) — no PE transposes.
     * conv1 runs as bf16 matmuls (1 cyc/row), 12 patch-pairs per PSUM tile;
       gelu(+b1) on ACT; conv2 as tiny bf16 matmuls producing [row,
       (patch, j)] directly in PSUM.
 - Box decode (per patch sub-range, on DVE): ds = relu(o1+b2[1]+7.5);
   an = dx+b2[0]+8p+7.5; lo/hi = clip(an -/+ ds, 0, 4095);
   alpha = lo-8p, beta = (hi-lo)/15 - 1.
 - Sampling identity: with j = 8p+s and w = alpha + beta*s in [-1, 1):
       out = X[j] + w*D1[j-1] + relu(w)*D2[j],
   D1[i] = X[i]-X[i-1] (shifted), D2[j] = D1[j+1]-D1[j].  All accesses are
   static strided views.  w is built as 16 per-s strided slabs
   (w[:, s::16] = beta*s + alpha, one STT per s).  The packed bf16 ops
   (relu/mult/mult/add/add) are split between DVE (2x/4x modes) and GPSIMD
   for engine balance.  Decode/interp runs on progressively finer sub-units
   at the start and end of the schedule to shorten pipeline fill/drain.
 - Output stored bf16, upcast on host.
"""

import numpy as np
import ml_dtypes

import concourse.bass as bass
import concourse.bacc as bacc
import concourse.mybir as mybir
from concourse.tile import TileContext
from concourse.bass_utils import run_bass_kernel_spmd

F32 = mybir.dt.float32
BF16 = mybir.dt.bfloat16
AF = mybir.ActivationFunctionType
OP = mybir.AluOpType
NPBF = ml_dtypes.bfloat16

# Problem constants
B, C, L = 32, 64, 4096
PS, STRIDE, PC, HID = 16, 8, 511, 64
NCORES = 8
BPC = B // NCORES            # batches per core
ROWS = BPC * C               # 256 (b,c) rows per core
NCHUNK = 2                   # chunks of 128 rows
NT = 256                     # patch-pair index t per chunk: p = 2t, 2t+1
TBLK = 12                    # t per conv1 PSUM tile (conv unit)
XOFF = 8                     # X[j] lives at xsb[:, XOFF + j]
XW = XOFF + L + 8            # padded row width

# decode/interp sub-unit boundaries (in patches within a 256-patch pair),
# per (chunk, pair): finer at schedule start (early GPSIMD fill) and end
# (short drain).
SUBS = {
    (0, 0): [0, 64, 128, 256],
    (0, 1): [0, 128, 256],
    (1, 0): [0, 128, 256],
    (1, 1): [0, 128, 192, 256],
}

C1 = 0.39217885179762646
C3 = -0.041966691335475

_CACHE = {}


def _consts(W1, b1, W2, b2):
    """Host-side packing of weights and constant tables."""
    W1 = np.asarray(W1, np.float32)
    b1 = np.asarray(b1, np.float32)
    W2 = np.asarray(W2, np.float32)
    b2 = np.asarray(b2, np.float32)

    # conv1 weight packs: pair t covers rows [16t, 16t+24) of the L axis;
    # within its 128-block the pair sits at row offset rho = 16*(t mod 8).
    # rho <= 96: single matmul with W1R{rho}; rho == 112: split into W1SA on
    # block A plus W1SB on block A+1, accumulated in PSUM.
    tens = {}
    for rho in range(0, 112, 16):
        full = np.zeros((128, 128), np.float32)
        full[rho:rho + 16, 0:64] = W1.T
        full[rho + 8:rho + 24, 64:128] = W1.T
        tens[f"W1R{rho}"] = full.astype(NPBF)
    w1sa = np.zeros((128, 128), np.float32)
    w1sa[112:128, 0:64] = W1.T
    w1sa[120:128, 64:128] = W1.T[0:8]       # odd patch s = 0..7
    tens["W1SA"] = w1sa.astype(NPBF)
    w1sb = np.zeros((128, 128), np.float32)
    w1sb[0:8, 64:128] = W1.T[8:16]          # odd patch s = 8..15
    tens["W1SB"] = w1sb.astype(NPBF)

    w2p = np.zeros((128, 4), np.float32)
    w2p[0:64, 0] = W2[0]
    w2p[0:64, 1] = W2[1]
    w2p[64:128, 2] = W2[0]
    w2p[64:128, 3] = W2[1]
    tens["W2P"] = w2p.astype(NPBF)
    tens["B1P"] = np.concatenate([b1, b1]).reshape(128, 1).astype(np.float32)

    p = np.arange(512, dtype=np.float32)
    tens["AREP"] = np.broadcast_to(8 * p + np.float32(7.5), (128, 512)).copy()
    tens["P8N"] = np.broadcast_to(-8 * p, (128, 512)).copy()

    # batch all constants into one bf16 + one f32 DMA (HWDGE dispatch is
    # serial at ~650ns each — 15 separate loads would stall startup)
    cb = np.concatenate([tens[k] for k in BF_ORDER], axis=1)
    cf = np.concatenate([tens[k].astype(np.float32) for k in F32_ORDER],
                        axis=1)
    packed = {"CB": np.ascontiguousarray(cb.astype(NPBF)),
              "CF": np.ascontiguousarray(cf.astype(np.float32))}

    scal = {
        "c_ds": float(np.float32(b2[1]) + np.float32(7.5)),
        "b20": float(np.float32(b2[0])),
    }
    return packed, scal


BF_ORDER = ([f"W1R{rho}" for rho in range(0, 112, 16)] +
            ["W1SA", "W1SB", "W2P"])
BF_COLS = [128] * 9 + [4]
F32_ORDER = ["B1P", "AREP", "P8N"]
F32_COLS = [1, 512, 512]


def _ap(tile_ap, col_off, dims):
    """Custom strided view of a 2D [128, F] tile: dims = [[step, count], ...]
    appended after the partition dim."""
    pstep = tile_ap.ap[0][0]
    npart = tile_ap.ap[0][1]
    return bass.AP(tile_ap.tensor, tile_ap.offset + col_off,
                   [[pstep, npart]] + [list(d) for d in dims])


def build(scal):
    nc = bacc.Bacc("TRN2", target_bir_lowering=False, debug=False)

    XS = nc.dram_tensor("XS", [ROWS, L], BF16, kind="ExternalInput")
    OUT = nc.dram_tensor("OUT", [ROWS, PC * PS], BF16, kind="ExternalOutput")
    CB = nc.dram_tensor("CB", [128, sum(BF_COLS)], BF16, kind="ExternalInput")
    CF = nc.dram_tensor("CF", [128, sum(F32_COLS)], F32, kind="ExternalInput")

    c_ds, b20 = scal["c_ds"], scal["b20"]

    with TileContext(nc) as tc:
        with tc.tile_pool(name="consts", bufs=1) as cpool, \
             tc.tile_pool(name="xbig", bufs=2) as xpool, \
             tc.tile_pool(name="work", bufs=2) as wpool, \
             tc.tile_pool(name="psum", bufs=2, space="PSUM") as ppool:

            xsbs, xts, d1ts, d2ts = [], [], [], []
            # ---- phase 1a: weights first, then chunk-0 X (HWDGE serial) ----
            cbt = cpool.tile([128, sum(BF_COLS)], BF16, tag="c_CB")
            nc.sync.dma_start(cbt[:, :], CB[:, :])
            cft = cpool.tile([128, sum(F32_COLS)], F32, tag="c_CF")
            ccol = {}
            c0 = 0
            for k, w in zip(BF_ORDER, BF_COLS):
                ccol[k] = (cbt, c0, w)
                c0 += w
            c0 = 0
            for k, w in zip(F32_ORDER, F32_COLS):
                ccol[k] = (cft, c0, w)
                c0 += w

            def cview(k, p0=0, pn=128, col0=0, ncols=None):
                tile, base, w = ccol[k]
                ap = tile[:, :]
                pstep = ap.ap[0][0]
                return bass.AP(ap.tensor,
                               ap.offset + p0 * pstep + base + col0,
                               [[pstep, pn],
                                [1, ncols if ncols is not None else w]])

            # pre-trigger the ACT Gelu table load off the critical path
            atl = cpool.tile([128, 4], F32, tag="atl")
            nc.scalar.activation(atl[:, 0:1], cbt[:, 0:1], AF.Gelu,
                                 bias=0.0, scale=1.0)

            for chunk in range(NCHUNK):
                r0 = chunk * 128
                xsb = xpool.tile([128, XW], BF16, tag="xsb")
                nc.gpsimd.memset(xsb[:, 0:XOFF], 0.0)
                nc.gpsimd.memset(xsb[:, XOFF + L:XW], 0.0)
                for half in range(2):
                    c0 = 2048 * half
                    nc.sync.dma_start(xsb[:, XOFF + c0:XOFF + c0 + 2048],
                                      XS[r0:r0 + 128, c0:c0 + 2048])
                # transposed copy (L-major 128-blocks) via DMA xbar
                xt = xpool.tile([128, 32 * 128], BF16, tag="xt")
                xtv = bass.AP(xt[:, :].tensor, xt[:, :].offset,
                              [[4096, 128], [128, 32], [1, 128]])
                nc.sync.dma_start_transpose(xtv, XS[r0:r0 + 128, 0:L])
                xsbs.append(xsb)
                xts.append(xt)
                if chunk == 0:
                    nc.sync.dma_start(cft[:, :], CF[:, :])

            def build_tables(chunk, d2_engine):
                xsb = xsbs[chunk]
                # d1t[:, i] = X[i] - X[i-1], i = 0..4096
                d1t = xpool.tile([128, L + 1], BF16, tag="d1t",
                                 name=f"d1t{chunk}")
                nc.vector.tensor_sub(d1t[:, 0:L + 1],
                                     xsb[:, XOFF:XOFF + L + 1],
                                     xsb[:, XOFF - 1:XOFF + L])
                # d2t[:, j] = D1[j] - D1[j-1] = d1t[j+1] - d1t[j]
                d2t = xpool.tile([128, L], BF16, tag="d2t",
                                 name=f"d2t{chunk}")
                d2_engine.tensor_sub(d2t[:, 0:L], d1t[:, 1:L + 1],
                                     d1t[:, 0:L])
                d1ts.append(d1t)
                d2ts.append(d2t)

            # chunk-0 d2 on GPSIMD (fills its startup hole); chunk-1 on DVE
            build_tables(0, nc.gpsimd)
            build_tables(1, nc.vector)

            # ---- phase 2: conv -> decode -> interp, pipelined ----
            def decode_interp(chunk, pair, offpt, lo, hi, alb, beb, wb):
                """Decode patches [lo,hi) of this pair, build w slabs, interp,
                and DMA the outputs."""
                r0 = chunk * 128
                xsb, d1t, d2t = xsbs[chunk], d1ts[chunk], d2ts[chunk]
                E = nc.vector
                pn = hi - lo
                p0 = 256 * pair + lo          # global patch base (per chunk)
                dxv = _ap(offpt[:, :], 2 * lo, [[2, pn]])
                o1v = _ap(offpt[:, :], 2 * lo + 1, [[2, pn]])
                dsb = wpool.tile([128, 256], F32, tag="dsb", bufs=2)
                E.tensor_scalar(dsb[:, 0:pn], o1v, c_ds, 0.0, OP.add, OP.max)
                anb = wpool.tile([128, 256], F32, tag="anb", bufs=2)
                E.scalar_tensor_tensor(anb[:, 0:pn], dxv, b20,
                                       cview("AREP", col0=p0, ncols=pn),
                                       OP.add, OP.add)
                lob = wpool.tile([128, 256], F32, tag="lob", bufs=2)
                E.scalar_tensor_tensor(lob[:, 0:pn], anb[:, 0:pn], 0.0,
                                       dsb[:, 0:pn], OP.add, OP.subtract)
                hib = wpool.tile([128, 256], F32, tag="hib", bufs=2)
                E.scalar_tensor_tensor(hib[:, 0:pn], anb[:, 0:pn], 0.0,
                                       dsb[:, 0:pn], OP.add, OP.add)
                E.tensor_scalar(lob[:, 0:pn], lob[:, 0:pn], 0.0, float(L - 1),
                                OP.max, OP.min)
                E.tensor_scalar(hib[:, 0:pn], hib[:, 0:pn], 0.0, float(L - 1),
                                OP.max, OP.min)
                E.scalar_tensor_tensor(alb[:, lo:hi], lob[:, 0:pn], 0.0,
                                       cview("P8N", col0=p0, ncols=pn),
                                       OP.add, OP.add)
                E.scalar_tensor_tensor(beb[:, lo:hi], hib[:, 0:pn], 0.0,
                                       lob[:, 0:pn], OP.add, OP.subtract)
                E.tensor_scalar(beb[:, lo:hi], beb[:, lo:hi], 1.0 / 15.0,
                                -1.0, OP.mult, OP.add)

                # w[:, 16p+s] = beta[p]*s + alpha[p], built by log-doubling:
                # w[0]=alpha; w[s..2s-1] = beta*s + w[0..s-1]
                w0 = _ap(wb[:, :], PS * lo, [[PS, pn]])
                nc.vector.tensor_copy(w0, _ap(alb[:, :], lo, [[1, pn]]))
                for sh in (1, 2, 4, 8):
                    wdst = _ap(wb[:, :], PS * lo + sh, [[PS, pn], [1, sh]])
                    wsrc = _ap(wb[:, :], PS * lo, [[PS, pn], [1, sh]])
                    bbc = _ap(beb[:, :], lo, [[1, pn], [0, sh]])
                    nc.vector.scalar_tensor_tensor(
                        wdst, bbc, float(sh), wsrc, OP.mult, OP.add)

                # interp in sub-units of <=128 patches (clip to PC)
                for ulo in range(lo, hi, 128):
                    uhi = min(ulo + 128, hi)
                    p0i = 256 * pair + ulo
                    pbn = min(uhi - ulo, PC - p0i)
                    n = PS * pbn
                    wv = _ap(wb[:, :], PS * ulo, [[PS, pbn], [1, PS]])
                    x1v = _ap(xsb[:, :], XOFF + 8 * p0i, [[8, pbn], [1, PS]])
                    d1mv = _ap(d1t[:, :], 8 * p0i, [[8, pbn], [1, PS]])
                    d2v = _ap(d2t[:, :], 8 * p0i, [[8, pbn], [1, PS]])

                    rb = wpool.tile([128, 2048], BF16, tag="rb", bufs=3,
                                    name=f"rb{chunk}_{pair}_{ulo}")
                    rbv = _ap(rb[:, :], 0, [[PS, pbn], [1, PS]])
                    last_u = (chunk == 1 and pair == 1 and uhi == 256)
                    late = (chunk == 1 and (pair == 1 or ulo >= 128))
                    if late:
                        # ACT is idle during the drain phase
                        nc.scalar.activation(rbv, wv, AF.Relu, bias=0.0,
                                             scale=1.0)
                    else:
                        nc.vector.tensor_scalar_max(rbv, wv, 0.0)
                    t1 = wpool.tile([128, 2048], BF16, tag="t1", bufs=3,
                                    name=f"t1{chunk}_{pair}_{ulo}")
                    t1v = _ap(t1[:, :], 0, [[PS, pbn], [1, PS]])
                    nc.vector.tensor_mul(t1v, wv, d1mv)
                    # T2 = relu(w)*D2 (in-place into rb); final unit's on
                    # DVE (GPSIMD's 4x-slower mult would sit on the drain
                    # critical path)
                    (nc.vector if last_u else nc.gpsimd).tensor_mul(
                        rbv, rbv, d2v)
                    # s1 = T1 + X[j]
                    nc.vector.tensor_add(t1v, t1v, x1v)
                    oap = bass.AP(OUT[:].tensor, r0 * PC * PS + p0i * PS,
                                  [[PC * PS, 128], [1, n]])
                    if chunk == 1 and pair == 1 and uhi == 256:
                        # drain tail: classic add + one HWDGE DMA is a
                        # shorter serial chain than two SWDGE DMAs
                        nc.vector.tensor_add(t1v, t1v, rbv)
                        nc.sync.dma_start(oap, t1[:, 0:n])
                    else:
                        # final add in the DMA engines: OUT = s1, then
                        # OUT += T2 (both on the gpsimd SWDGE queue =>
                        # ordered)
                        nc.gpsimd.dma_start(oap, t1[:, 0:n])
                        nc.gpsimd.dma_start(oap, rb[:, 0:n], accum_op=OP.add)

            for chunk in range(NCHUNK):
                xt = xts[chunk]
                offpts = {}
                albs = {}
                # conv units of TBLK pairs; decode sub-units as their patch
                # ranges complete
                tstarts = list(range(0, NT, TBLK))
                done_subs = set()
                for t0 in tstarts:
                    tn = min(TBLK, NT - t0)
                    pt = ppool.tile([128, TBLK * 128], F32, tag="pt", bufs=2)
                    for q in range(tn):
                        t = t0 + q
                        blkA, rho = divmod(16 * t, 128)
                        dst = pt[:, 128 * q:128 * (q + 1)]
                        if rho <= 96:
                            nc.tensor.matmul(
                                dst, cview(f"W1R{rho}"),
                                xt[:, 128 * blkA:128 * (blkA + 1)],
                                start=True, stop=True)
                        elif t == NT - 1:
                            # patch 511 (discarded) needs block 32; skip
                            nc.tensor.matmul(
                                dst, cview("W1SA", p0=64, pn=64),
                                xt[64:128, 128 * blkA:128 * (blkA + 1)],
                                start=True, stop=True)
                        else:
                            nc.tensor.matmul(
                                dst, cview("W1SA", p0=64, pn=64),
                                xt[64:128, 128 * blkA:128 * (blkA + 1)],
                                start=True, stop=False)
                            nc.tensor.matmul(
                                dst, cview("W1SB", p0=0, pn=8),
                                xt[0:8, 128 * (blkA + 1):128 * (blkA + 2)],
                                start=False, stop=True)
                    hsb = wpool.tile([128, TBLK * 128], BF16, tag="hsb",
                                     bufs=4)
                    nc.scalar.activation(hsb[:, 0:128 * tn],
                                         pt[:, 0:128 * tn],
                                         AF.Gelu, bias=cview("B1P"),
                                         scale=1.0)
                    for q in range(tn):
                        t = t0 + q
                        pair = t // 128
                        if pair not in offpts:
                            offpts[pair] = ppool.tile(
                                [128, 512], F32, tag="offpt", bufs=2,
                                name=f"offpt{chunk}_{pair}")
                            alb = wpool.tile([128, 256], F32, tag="alb",
                                             bufs=2, name=f"alb{chunk}{pair}")
                            beb = wpool.tile([128, 256], F32, tag="beb",
                                             bufs=2, name=f"beb{chunk}{pair}")
                            wbt = wpool.tile([128, 4096], BF16, tag="wbt",
                                             bufs=2, name=f"wbt{chunk}{pair}")
                            albs[pair] = (alb, beb, wbt)
                        col = 4 * (t - 128 * pair)
                        nc.tensor.matmul(
                            offpts[pair][:, col:col + 4],
                            hsb[:, 128 * q:128 * (q + 1)],
                            cview("W2P"),
                            start=True, stop=True)

                    # emit any decode sub-units now complete
                    t_done = t0 + tn          # pairs fully conv'd below this
                    for pair in (0, 1):
                        subs = SUBS[(chunk, pair)]
                        for si in range(len(subs) - 1):
                            key = (pair, si)
                            if key in done_subs:
                                continue
                            # need conv2 for patches < 256*pair + subs[si+1],
                            # i.e. t < 128*pair + subs[si+1]/2
                            if 2 * t_done >= 256 * pair + subs[si + 1]:
                                alb, beb, wbt = albs[pair]
                                decode_interp(chunk, pair, offpts[pair],
                                              subs[si], subs[si + 1],
                                              alb, beb, wbt)
                                done_subs.add(key)
    nc.finalize()
    return nc


def kernel(X, W1, b1, W2, b2):
    X = np.ascontiguousarray(np.asarray(X, np.float32))
    tens, scal = _consts(W1, b1, W2, b2)
    key = tuple(sorted(scal.items()))
    if _CACHE.get("key") != key:
        _CACHE["nc"] = build(scal)
        _CACHE["key"] = key
    nc = _CACHE["nc"]

    Xb = X.astype(NPBF)
    in_maps = []
    for i in range(NCORES):
        m = {"XS": Xb[BPC * i:BPC * (i + 1)].reshape(ROWS, L)}
        m.update(tens)
        in_maps.append(m)

    res = run_bass_kernel_spmd(nc, in_maps, core_ids=list(range(NCORES)))
    out = np.concatenate(
        [np.asarray(res.results[i]["OUT"]).astype(np.float32)
         .reshape(BPC, C, PC, PS) for i in range(NCORES)], axis=0)
    return out
